# revision 21
# baseline (speedup 1.0000x reference)
"""Trainium2 Bass kernel for the NTN problem.

out[b,k,q,a] = sigmoid( q[b,q,:] @ w[k] @ da[b,a,:]
                        + Vq[k]@q[b,q,:] + Vd[k]@da[b,a,:] + b[k] )

B=64, K=16, Q=A=D=256.  Sharding: data-parallel over batch B across the
8 NeuronCores (8 batches per core); w/V/b replicated.

Per core, per (k, batch-pair):
  MM1 (TensorE, fp16): tmp[e, q|q'] = sum_d w[k,d,e]^T qT[d, q|q']   (N=512)
  DVE: tmp PSUM->SBUF (fp16) with per-partition bias +Vd[k,e] (folds Vd@da)
  MM2 (TensorE, fp16): out[q, a] = sum_e tmp[e,q]^T daT[e, a]
  ScalarE: sigmoid(psum + bias mq[b,k,q]) where mq = Vq@q + b (host-prepped),
           written into a per-(b,qtile) collect tile covering all 16 k
  One 2 MB DMA per (b, qtile) collect tile -> 16 output stores total.
"""

import os
import sys
import types
from contextlib import ExitStack

if "/opt/trn_rl_repo" not in sys.path:
    sys.path.insert(0, "/opt/trn_rl_repo")

import numpy as np

import concourse.bass as bass
import concourse.tile as tile
from concourse import bacc, bass_utils, mybir

F32 = mybir.dt.float32
F16 = mybir.dt.float16
SIG = mybir.ActivationFunctionType.Sigmoid

NCORES = 8
B, Q, A, D, K = 64, 256, 256, 256, 16
E = D
BL = B // NCORES


def _install_profshim():
    """Provide antenv.axon_hooks so trace=True works under axon (best-effort)."""
    try:
        if "antenv.axon_hooks" in sys.modules:
            return True
        import antenv

        mod = types.ModuleType("antenv.axon_hooks")
        holder = {}
        mod.set_axon_ntff_profile_hook = lambda h: holder.__setitem__("h", h)
        mod.get_axon_ntff_profile_hook = lambda: holder.get("h")
        sys.modules["antenv.axon_hooks"] = mod
        antenv.axon_hooks = mod
        from trn_agent_boot.trn_boot import _ntff_profile_via_ctypes

        hook = _ntff_profile_via_ctypes("/opt/axon/libaxon_pjrt.so")
        if hook is None:
            return False
        mod.set_axon_ntff_profile_hook(hook)
        return True
    except Exception:
        return False


def _build_ntn(tc: tile.TileContext, ctx: ExitStack, aps: dict):
    nc = tc.nc
    DC, ET, QT = D // 128, E // 128, Q // 128
    qt, dat, w, vdt, mq, out = (aps[n] for n in ("qt", "dat", "w", "vdt", "mq", "out"))

    w_pool = ctx.enter_context(tc.tile_pool(name="w", bufs=1))
    const_pool = ctx.enter_context(tc.tile_pool(name="const", bufs=1))
    q_pool = ctx.enter_context(tc.tile_pool(name="q", bufs=3))
    da_pool = ctx.enter_context(tc.tile_pool(name="da", bufs=3))
    tmp_pool = ctx.enter_context(tc.tile_pool(name="tmp", bufs=4))
    out_pool = ctx.enter_context(tc.tile_pool(name="out", bufs=24))
    ptmp_pool = ctx.enter_context(tc.tile_pool(name="ptmp", bufs=3, space="PSUM"))
    pout_pool = ctx.enter_context(tc.tile_pool(name="pout", bufs=5, space="PSUM"))

    # First batch-pair's activations first, so MM1(bp0,k=0) can start early;
    # then per-k w tiles, then the small constants.
    act_tiles = {}

    # PE warm-up: dummy matmuls on a zeroed scratch tile while input DMAs are
    # in flight, so HAM reaches full clock before the real stream starts.
    warm_pool = ctx.enter_context(tc.tile_pool(name="warm", bufs=1))
    scratch = warm_pool.tile([128, 512], F16)
    nc.gpsimd.memset(scratch[:], 0.0)
    pwarm = ptmp_pool.tile([128, 512], F32, name="pwarm", tag="pt")
    for _ in range(7):
        nc.tensor.matmul(pwarm[:], lhsT=scratch[:, 0:128], rhs=scratch[:],
                         start=True, stop=True)

    def load_pair(bp, first=False):
        b0, b1 = 2 * bp, 2 * bp + 1
        q2 = q_pool.tile([128, DC, 2 * Q], F16, name=f"q2_{bp}", tag="q2")
        nc.sync.dma_start(q2[:, :, 0:Q], qt[b0])
        nc.sync.dma_start(q2[:, :, Q:2 * Q], qt[b1])
        if first:
            return (q2, b0, b1)
        da2 = da_pool.tile([128, ET, 2 * A], F16, name=f"da2_{bp}", tag="da2")
        nc.sync.dma_start(da2[:, :, 0:A], dat[b0])
        nc.sync.dma_start(da2[:, :, A:2 * A], dat[b1])
        act_tiles[bp] = (q2, da2)

    def load_wk(k):
        wk = w_pool.tile([128, DC, E], F16, name=f"wk{k}", tag=f"wk{k}")
        nc.sync.dma_start(wk[:], w[k].rearrange("(dc p) e -> p dc e", p=128))
        return wk

    q2_0, b0_0, b1_0 = load_pair(0, first=True)
    mq_sb = const_pool.tile([128, QT, BL, K], F32)
    nc.sync.dma_start(mq_sb[:], mq.rearrange("t p b k -> p t b k"))
    w_sb = {}
    w_sb[0] = load_wk(0)
    da2_0 = da_pool.tile([128, ET, 2 * A], F16, name="da2_0", tag="da2")
    nc.sync.dma_start(da2_0[:, :, 0:A], dat[b0_0])
    nc.sync.dma_start(da2_0[:, :, A:2 * A], dat[b1_0])
    act_tiles[0] = (q2_0, da2_0)
    w_sb[1] = load_wk(1)
    vdt_sb = const_pool.tile([128, ET, 128], F32)
    nc.sync.dma_start(vdt_sb[:], vdt.rearrange("et p k -> p et k"))
    for k in range(2, K):
        w_sb[k] = load_wk(k)

    for bp in range(BL // 2):
        b0, b1 = 2 * bp, 2 * bp + 1
        if bp not in act_tiles:
            load_pair(bp)
        q2, da2 = act_tiles.pop(bp)

        # per-(b, qtile, k-chunk) collect tiles; finer chunks for the last
        # batch-pair so the final store flush is small
        KH = (K // 4) if bp < BL // 2 - 1 else (K // 8)
        coll = {(h, qt_i, kh): out_pool.tile([128, KH, A], F32, name="coll", tag="coll")
                for h in (0, 1) for qt_i in range(QT) for kh in range(K // KH)}

        for k in range(K):
            ptmps = []
            for et in range(ET):
                pt = ptmp_pool.tile([128, 2 * Q], F32)
                for dc in range(DC):
                    nc.tensor.matmul(
                        pt[:],
                        lhsT=w_sb[k][:, dc, et * 128:(et + 1) * 128],
                        rhs=q2[:, dc, :],
                        start=(dc == 0),
                        stop=(dc == DC - 1),
                    )
                ptmps.append(pt)
            tmp = tmp_pool.tile([128, ET, 2 * Q], F16)
            for et in range(ET):
                nc.vector.tensor_scalar_add(
                    tmp[:, et, :], ptmps[et][:], vdt_sb[:, et, k:k + 1]
                )
            for h, b in ((0, b0), (1, b1)):
                for qt_i in range(QT):
                    po = pout_pool.tile([128, A], F32)
                    for et in range(ET):
                        nc.tensor.matmul(
                            po[:],
                            lhsT=tmp[:, et, h * Q + qt_i * 128: h * Q + (qt_i + 1) * 128],
                            rhs=da2[:, et, h * A:(h + 1) * A],
                            start=(et == 0),
                            stop=(et == ET - 1),
                        )
                    nc.scalar.activation(
                        coll[(h, qt_i, k // KH)][:, k % KH, :], po[:], SIG,
                        bias=mq_sb[:, qt_i, b, k:k + 1],
                    )
            if (k + 1) % KH == 0:
                kh = k // KH
                store_eng = nc.sync
                for h, b in ((0, b0), (1, b1)):
                    for qt_i in range(QT):
                        store_eng.dma_start(
                            out[b, kh * KH:(kh + 1) * KH,
                                qt_i * 128:(qt_i + 1) * 128, :].rearrange("k p a -> p k a"),
                            coll[(h, qt_i, kh)][:],
                        )


_COMPILED = None


def _get_compiled():
    global _COMPILED
    if _COMPILED is not None:
        return _COMPILED
    nc = bacc.Bacc("TRN2", target_bir_lowering=False, debug=False, num_devices=NCORES)
    aps = {
        "qt": nc.dram_tensor("qt", [BL, 128, D // 128, Q], F16, kind="ExternalInput").ap(),
        "dat": nc.dram_tensor("dat", [BL, 128, E // 128, A], F16, kind="ExternalInput").ap(),
        "w": nc.dram_tensor("w", [K, D, E], F16, kind="ExternalInput").ap(),
        "vdt": nc.dram_tensor("vdt", [E // 128, 128, 128], F32, kind="ExternalInput").ap(),
        "mq": nc.dram_tensor("mq", [Q // 128, 128, BL, K], F32, kind="ExternalInput").ap(),
        "out": nc.dram_tensor("out", [BL, K, Q, A], F32, kind="ExternalOutput").ap(),
    }
    with tile.TileContext(nc) as tc:
        with ExitStack() as ctx:
            _build_ntn(tc, ctx, aps)
    nc.compile()
    _COMPILED = nc
    return nc


def kernel(batch_q_em, batch_da_em, w, V, b):
    q = np.ascontiguousarray(np.asarray(batch_q_em, dtype=np.float32))
    da = np.ascontiguousarray(np.asarray(batch_da_em, dtype=np.float32))
    w = np.ascontiguousarray(np.asarray(w, dtype=np.float32))
    V = np.ascontiguousarray(np.asarray(V, dtype=np.float32))
    b = np.asarray(b, dtype=np.float32).reshape(-1)

    # packed to SBUF layout [b, p, dc, q] so each load is 128 x 1KB descriptors
    qt = np.ascontiguousarray(
        q.transpose(0, 2, 1).reshape(B, D // 128, 128, Q).transpose(0, 2, 1, 3)
    ).astype(np.float16)                                  # [B, 128, DC, Q]
    dat = np.ascontiguousarray(
        da.transpose(0, 2, 1).reshape(B, E // 128, 128, A).transpose(0, 2, 1, 3)
    ).astype(np.float16)                                  # [B, 128, ET, A]
    w16 = w.astype(np.float16)
    vdt_cols = np.ascontiguousarray(V[:, D:].T)          # [E, K]
    vdt = np.zeros((E // 128, 128, 128), dtype=np.float32)
    vdt[:, :, :K] = vdt_cols.reshape(E // 128, 128, K)
    # mq[b,q,k] = q[b] @ Vq^T + bias
    mqT = q @ V[:, :D].T + b[None, None, :]              # [B, Q, K]

    nc = _get_compiled()
    in_maps = []
    for c in range(NCORES):
        s = slice(c * BL, (c + 1) * BL)
        mq_shard = np.ascontiguousarray(
            mqT[s].reshape(BL, Q // 128, 128, K).transpose(1, 2, 0, 3)
        )  # [QT, 128, BL, K]
        in_maps.append({
            "qt": np.ascontiguousarray(qt[s]),
            "dat": np.ascontiguousarray(dat[s]),
            "w": w16,
            "vdt": vdt,
            "mq": mq_shard,
        })

    trace = bool(int(os.environ.get("NTN_TRACE", "0"))) and _install_profshim()
    res = bass_utils.run_bass_kernel_spmd(
        nc, in_maps, core_ids=list(range(NCORES)), trace=trace
    )
    if trace and res.exec_time_ns is not None:
        print(f"HW exec time: {res.exec_time_ns} ns")
    out = np.concatenate([r["out"] for r in res.results], axis=0)
    return out


# revision 22
# speedup vs baseline: 1.0132x; 1.0132x over previous
"""Trainium2 Bass kernel for the NTN problem.

out[b,k,q,a] = sigmoid( q[b,q,:] @ w[k] @ da[b,a,:]
                        + Vq[k]@q[b,q,:] + Vd[k]@da[b,a,:] + b[k] )

B=64, K=16, Q=A=D=256.  Sharding: data-parallel over batch B across the
8 NeuronCores (8 batches per core); w/V/b replicated.

Per core, per (k, batch-pair):
  MM1 (TensorE, fp16): tmp[e, q|q'] = sum_d w[k,d,e]^T qT[d, q|q']   (N=512)
  DVE: tmp PSUM->SBUF (fp16) with per-partition bias +Vd[k,e] (folds Vd@da)
  MM2 (TensorE, fp16): out[q, a] = sum_e tmp[e,q]^T daT[e, a]
  ScalarE: sigmoid(psum + bias mq[b,k,q]) where mq = Vq@q + b (host-prepped),
           written into a per-(b,qtile) collect tile covering all 16 k
  One 2 MB DMA per (b, qtile) collect tile -> 16 output stores total.
"""

import os
import sys
import types
from contextlib import ExitStack

if "/opt/trn_rl_repo" not in sys.path:
    sys.path.insert(0, "/opt/trn_rl_repo")

import numpy as np

import concourse.bass as bass
import concourse.tile as tile
from concourse import bacc, bass_utils, mybir

F32 = mybir.dt.float32
F16 = mybir.dt.float16
SIG = mybir.ActivationFunctionType.Sigmoid

NCORES = 8
B, Q, A, D, K = 64, 256, 256, 256, 16
E = D
BL = B // NCORES


def _install_profshim():
    """Provide antenv.axon_hooks so trace=True works under axon (best-effort)."""
    try:
        if "antenv.axon_hooks" in sys.modules:
            return True
        import antenv

        mod = types.ModuleType("antenv.axon_hooks")
        holder = {}
        mod.set_axon_ntff_profile_hook = lambda h: holder.__setitem__("h", h)
        mod.get_axon_ntff_profile_hook = lambda: holder.get("h")
        sys.modules["antenv.axon_hooks"] = mod
        antenv.axon_hooks = mod
        from trn_agent_boot.trn_boot import _ntff_profile_via_ctypes

        hook = _ntff_profile_via_ctypes("/opt/axon/libaxon_pjrt.so")
        if hook is None:
            return False
        mod.set_axon_ntff_profile_hook(hook)
        return True
    except Exception:
        return False


def _build_ntn(tc: tile.TileContext, ctx: ExitStack, aps: dict):
    nc = tc.nc
    DC, ET, QT = D // 128, E // 128, Q // 128
    qt, dat, w, vdt, mq, out = (aps[n] for n in ("qt", "dat", "w", "vdt", "mq", "out"))

    w_pool = ctx.enter_context(tc.tile_pool(name="w", bufs=1))
    const_pool = ctx.enter_context(tc.tile_pool(name="const", bufs=1))
    q_pool = ctx.enter_context(tc.tile_pool(name="q", bufs=3))
    da_pool = ctx.enter_context(tc.tile_pool(name="da", bufs=3))
    tmp_pool = ctx.enter_context(tc.tile_pool(name="tmp", bufs=4))
    out_pool = ctx.enter_context(tc.tile_pool(name="out", bufs=24))
    ptmp_pool = ctx.enter_context(tc.tile_pool(name="ptmp", bufs=3, space="PSUM"))
    pout_pool = ctx.enter_context(tc.tile_pool(name="pout", bufs=5, space="PSUM"))

    # First batch-pair's activations first, so MM1(bp0,k=0) can start early;
    # then per-k w tiles, then the small constants.
    act_tiles = {}

    # PE warm-up: dummy matmuls on a zeroed scratch tile while input DMAs are
    # in flight, so HAM reaches full clock before the real stream starts.
    warm_pool = ctx.enter_context(tc.tile_pool(name="warm", bufs=1))
    scratch = warm_pool.tile([128, 512], F16)
    nc.gpsimd.memset(scratch[:], 0.0)
    pwarm = ptmp_pool.tile([128, 512], F32, name="pwarm", tag="pt")
    for _ in range(9):
        nc.tensor.matmul(pwarm[:], lhsT=scratch[:, 0:128], rhs=scratch[:],
                         start=True, stop=True)

    def load_pair(bp, first=False):
        b0, b1 = 2 * bp, 2 * bp + 1
        q2 = q_pool.tile([128, DC, 2 * Q], F16, name=f"q2_{bp}", tag="q2")
        nc.sync.dma_start(q2[:, :, 0:Q], qt[b0])
        nc.sync.dma_start(q2[:, :, Q:2 * Q], qt[b1])
        if first:
            return (q2, b0, b1)
        da2 = da_pool.tile([128, ET, 2 * A], F16, name=f"da2_{bp}", tag="da2")
        nc.sync.dma_start(da2[:, :, 0:A], dat[b0])
        nc.sync.dma_start(da2[:, :, A:2 * A], dat[b1])
        act_tiles[bp] = (q2, da2)

    def load_wk(k):
        wk = w_pool.tile([128, DC, E], F16, name=f"wk{k}", tag=f"wk{k}")
        nc.sync.dma_start(wk[:], w[k].rearrange("(dc p) e -> p dc e", p=128))
        return wk

    q2_0, b0_0, b1_0 = load_pair(0, first=True)
    mq_sb = const_pool.tile([128, QT, BL, K], F32)
    nc.sync.dma_start(mq_sb[:], mq.rearrange("t p b k -> p t b k"))
    w_sb = {}
    w_sb[0] = load_wk(0)
    da2_0 = da_pool.tile([128, ET, 2 * A], F16, name="da2_0", tag="da2")
    nc.sync.dma_start(da2_0[:, :, 0:A], dat[b0_0])
    nc.sync.dma_start(da2_0[:, :, A:2 * A], dat[b1_0])
    act_tiles[0] = (q2_0, da2_0)
    w_sb[1] = load_wk(1)
    vdt_sb = const_pool.tile([128, ET, 128], F32)
    nc.sync.dma_start(vdt_sb[:], vdt.rearrange("et p k -> p et k"))
    for k in range(2, K):
        w_sb[k] = load_wk(k)

    NBP = BL // 2
    for bp in range(NBP):
        b0, b1 = 2 * bp, 2 * bp + 1
        if bp not in act_tiles:
            load_pair(bp)
        if bp + 1 < NBP and bp + 1 not in act_tiles:
            load_pair(bp + 1)
        q2, da2 = act_tiles.pop(bp)

        # per-(b, qtile, k-chunk) collect tiles; finer chunks for the last
        # batch-pair so the final store flush is small
        KH = (K // 4) if bp < NBP - 1 else (K // 8)
        coll = {(h, qt_i, kh): out_pool.tile([128, KH, A], F32, name="coll", tag="coll")
                for h in (0, 1) for qt_i in range(QT) for kh in range(K // KH)}

        for k in range(K):
            ptmps = []
            for et in range(ET):
                pt = ptmp_pool.tile([128, 2 * Q], F32)
                for dc in range(DC):
                    nc.tensor.matmul(
                        pt[:],
                        lhsT=w_sb[k][:, dc, et * 128:(et + 1) * 128],
                        rhs=q2[:, dc, :],
                        start=(dc == 0),
                        stop=(dc == DC - 1),
                    )
                ptmps.append(pt)
            tmp = tmp_pool.tile([128, ET, 2 * Q], F16)
            for et in range(ET):
                nc.vector.tensor_scalar_add(
                    tmp[:, et, :], ptmps[et][:], vdt_sb[:, et, k:k + 1]
                )
            for h, b in ((0, b0), (1, b1)):
                for qt_i in range(QT):
                    po = pout_pool.tile([128, A], F32)
                    for et in range(ET):
                        nc.tensor.matmul(
                            po[:],
                            lhsT=tmp[:, et, h * Q + qt_i * 128: h * Q + (qt_i + 1) * 128],
                            rhs=da2[:, et, h * A:(h + 1) * A],
                            start=(et == 0),
                            stop=(et == ET - 1),
                        )
                    nc.scalar.activation(
                        coll[(h, qt_i, k // KH)][:, k % KH, :], po[:], SIG,
                        bias=mq_sb[:, qt_i, b, k:k + 1],
                    )
            if (k + 1) % KH == 0:
                kh = k // KH
                store_eng = nc.sync
                for h, b in ((0, b0), (1, b1)):
                    for qt_i in range(QT):
                        store_eng.dma_start(
                            out[b, kh * KH:(kh + 1) * KH,
                                qt_i * 128:(qt_i + 1) * 128, :].rearrange("k p a -> p k a"),
                            coll[(h, qt_i, kh)][:],
                        )


_COMPILED = None


def _get_compiled():
    global _COMPILED
    if _COMPILED is not None:
        return _COMPILED
    nc = bacc.Bacc("TRN2", target_bir_lowering=False, debug=False, num_devices=NCORES)
    aps = {
        "qt": nc.dram_tensor("qt", [BL, 128, D // 128, Q], F16, kind="ExternalInput").ap(),
        "dat": nc.dram_tensor("dat", [BL, 128, E // 128, A], F16, kind="ExternalInput").ap(),
        "w": nc.dram_tensor("w", [K, D, E], F16, kind="ExternalInput").ap(),
        "vdt": nc.dram_tensor("vdt", [E // 128, 128, 128], F32, kind="ExternalInput").ap(),
        "mq": nc.dram_tensor("mq", [Q // 128, 128, BL, K], F32, kind="ExternalInput").ap(),
        "out": nc.dram_tensor("out", [BL, K, Q, A], F32, kind="ExternalOutput").ap(),
    }
    with tile.TileContext(nc) as tc:
        with ExitStack() as ctx:
            _build_ntn(tc, ctx, aps)
    nc.compile()
    _COMPILED = nc
    return nc


def kernel(batch_q_em, batch_da_em, w, V, b):
    q = np.ascontiguousarray(np.asarray(batch_q_em, dtype=np.float32))
    da = np.ascontiguousarray(np.asarray(batch_da_em, dtype=np.float32))
    w = np.ascontiguousarray(np.asarray(w, dtype=np.float32))
    V = np.ascontiguousarray(np.asarray(V, dtype=np.float32))
    b = np.asarray(b, dtype=np.float32).reshape(-1)

    # packed to SBUF layout [b, p, dc, q] so each load is 128 x 1KB descriptors
    qt = np.ascontiguousarray(
        q.transpose(0, 2, 1).reshape(B, D // 128, 128, Q).transpose(0, 2, 1, 3)
    ).astype(np.float16)                                  # [B, 128, DC, Q]
    dat = np.ascontiguousarray(
        da.transpose(0, 2, 1).reshape(B, E // 128, 128, A).transpose(0, 2, 1, 3)
    ).astype(np.float16)                                  # [B, 128, ET, A]
    w16 = w.astype(np.float16)
    vdt_cols = np.ascontiguousarray(V[:, D:].T)          # [E, K]
    vdt = np.zeros((E // 128, 128, 128), dtype=np.float32)
    vdt[:, :, :K] = vdt_cols.reshape(E // 128, 128, K)
    # mq[b,q,k] = q[b] @ Vq^T + bias
    mqT = q @ V[:, :D].T + b[None, None, :]              # [B, Q, K]

    nc = _get_compiled()
    in_maps = []
    for c in range(NCORES):
        s = slice(c * BL, (c + 1) * BL)
        mq_shard = np.ascontiguousarray(
            mqT[s].reshape(BL, Q // 128, 128, K).transpose(1, 2, 0, 3)
        )  # [QT, 128, BL, K]
        in_maps.append({
            "qt": np.ascontiguousarray(qt[s]),
            "dat": np.ascontiguousarray(dat[s]),
            "w": w16,
            "vdt": vdt,
            "mq": mq_shard,
        })

    trace = bool(int(os.environ.get("NTN_TRACE", "0"))) and _install_profshim()
    res = bass_utils.run_bass_kernel_spmd(
        nc, in_maps, core_ids=list(range(NCORES)), trace=trace
    )
    if trace and res.exec_time_ns is not None:
        print(f"HW exec time: {res.exec_time_ns} ns")
    out = np.concatenate([r["out"] for r in res.results], axis=0)
    return out


# revision 23
# speedup vs baseline: 1.0185x; 1.0053x over previous
"""Trainium2 Bass kernel for the NTN problem.

out[b,k,q,a] = sigmoid( q[b,q,:] @ w[k] @ da[b,a,:]
                        + Vq[k]@q[b,q,:] + Vd[k]@da[b,a,:] + b[k] )

B=64, K=16, Q=A=D=256.  Sharding: data-parallel over batch B across the
8 NeuronCores (8 batches per core); w/V/b replicated.

Per core, per (k, batch-pair):
  MM1 (TensorE, fp16): tmp[e, q|q'] = sum_d w[k,d,e]^T qT[d, q|q']   (N=512)
  DVE: tmp PSUM->SBUF (fp16) with per-partition bias +Vd[k,e] (folds Vd@da)
  MM2 (TensorE, fp16): out[q, a] = sum_e tmp[e,q]^T daT[e, a]
  ScalarE: sigmoid(psum + bias mq[b,k,q]) where mq = Vq@q + b (host-prepped),
           written into a per-(b,qtile) collect tile covering all 16 k
  One 2 MB DMA per (b, qtile) collect tile -> 16 output stores total.
"""

import os
import sys
import types
from contextlib import ExitStack

if "/opt/trn_rl_repo" not in sys.path:
    sys.path.insert(0, "/opt/trn_rl_repo")

import numpy as np

import concourse.bass as bass
import concourse.tile as tile
from concourse import bacc, bass_utils, mybir

F32 = mybir.dt.float32
F16 = mybir.dt.float16
SIG = mybir.ActivationFunctionType.Sigmoid

NCORES = 8
B, Q, A, D, K = 64, 256, 256, 256, 16
E = D
BL = B // NCORES


def _install_profshim():
    """Provide antenv.axon_hooks so trace=True works under axon (best-effort)."""
    try:
        if "antenv.axon_hooks" in sys.modules:
            return True
        import antenv

        mod = types.ModuleType("antenv.axon_hooks")
        holder = {}
        mod.set_axon_ntff_profile_hook = lambda h: holder.__setitem__("h", h)
        mod.get_axon_ntff_profile_hook = lambda: holder.get("h")
        sys.modules["antenv.axon_hooks"] = mod
        antenv.axon_hooks = mod
        from trn_agent_boot.trn_boot import _ntff_profile_via_ctypes

        hook = _ntff_profile_via_ctypes("/opt/axon/libaxon_pjrt.so")
        if hook is None:
            return False
        mod.set_axon_ntff_profile_hook(hook)
        return True
    except Exception:
        return False


def _build_ntn(tc: tile.TileContext, ctx: ExitStack, aps: dict):
    nc = tc.nc
    DC, ET, QT = D // 128, E // 128, Q // 128
    qt, dat, w, vdt, mq, out = (aps[n] for n in ("qt", "dat", "w", "vdt", "mq", "out"))

    w_pool = ctx.enter_context(tc.tile_pool(name="w", bufs=1))
    const_pool = ctx.enter_context(tc.tile_pool(name="const", bufs=1))
    q_pool = ctx.enter_context(tc.tile_pool(name="q", bufs=3))
    da_pool = ctx.enter_context(tc.tile_pool(name="da", bufs=3))
    tmp_pool = ctx.enter_context(tc.tile_pool(name="tmp", bufs=4))
    out_pool = ctx.enter_context(tc.tile_pool(name="out", bufs=24))
    ptmp_pool = ctx.enter_context(tc.tile_pool(name="ptmp", bufs=3, space="PSUM"))
    pout_pool = ctx.enter_context(tc.tile_pool(name="pout", bufs=5, space="PSUM"))

    # First batch-pair's activations first, so MM1(bp0,k=0) can start early;
    # then per-k w tiles, then the small constants.
    act_tiles = {}

    # PE warm-up: dummy matmuls on a zeroed scratch tile while input DMAs are
    # in flight, so HAM reaches full clock before the real stream starts.
    warm_pool = ctx.enter_context(tc.tile_pool(name="warm", bufs=1))
    scratch = warm_pool.tile([128, 512], F16)
    nc.gpsimd.memset(scratch[:], 0.0)
    pwarm = ptmp_pool.tile([128, 512], F32, name="pwarm", tag="pt")
    for _ in range(9):
        nc.tensor.matmul(pwarm[:], lhsT=scratch[:, 0:128], rhs=scratch[:],
                         start=True, stop=True)

    def load_pair(bp, first=False):
        b0, b1 = 2 * bp, 2 * bp + 1
        q2 = q_pool.tile([128, DC, 2 * Q], F16, name=f"q2_{bp}", tag="q2")
        nc.sync.dma_start(q2[:, :, 0:Q], qt[b0])
        nc.sync.dma_start(q2[:, :, Q:2 * Q], qt[b1])
        if first:
            return (q2, b0, b1)
        da2 = da_pool.tile([128, ET, 2 * A], F16, name=f"da2_{bp}", tag="da2")
        nc.sync.dma_start(da2[:, :, 0:A], dat[b0])
        nc.sync.dma_start(da2[:, :, A:2 * A], dat[b1])
        act_tiles[bp] = (q2, da2)

    def load_wk(k):
        wk = w_pool.tile([128, DC, E], F16, name=f"wk{k}", tag=f"wk{k}")
        nc.sync.dma_start(wk[:], w[k].rearrange("(dc p) e -> p dc e", p=128))
        return wk

    # critical first loads on ScalarE's HWDGE stream (its framework preamble
    # ends ~1.7us before SP's, so these issue earlier)
    b0_0, b1_0 = 0, 1
    q2_0 = q_pool.tile([128, DC, 2 * Q], F16, name="q2_0", tag="q2")
    nc.scalar.dma_start(q2_0[:, :, 0:Q], qt[b0_0])
    nc.scalar.dma_start(q2_0[:, :, Q:2 * Q], qt[b1_0])
    w_sb = {}
    wk0 = w_pool.tile([128, DC, E], F16, name="wk0", tag="wk0")
    nc.scalar.dma_start(wk0[:], w[0].rearrange("(dc p) e -> p dc e", p=128))
    w_sb[0] = wk0
    mq_sb = const_pool.tile([128, QT, BL, K], F32)
    nc.sync.dma_start(mq_sb[:], mq.rearrange("t p b k -> p t b k"))
    da2_0 = da_pool.tile([128, ET, 2 * A], F16, name="da2_0", tag="da2")
    nc.sync.dma_start(da2_0[:, :, 0:A], dat[b0_0])
    nc.sync.dma_start(da2_0[:, :, A:2 * A], dat[b1_0])
    act_tiles[0] = (q2_0, da2_0)
    w_sb[1] = load_wk(1)
    vdt_sb = const_pool.tile([128, ET, 128], F32)
    nc.sync.dma_start(vdt_sb[:], vdt.rearrange("et p k -> p et k"))
    for k in range(2, K):
        w_sb[k] = load_wk(k)

    NBP = BL // 2
    for bp in range(NBP):
        b0, b1 = 2 * bp, 2 * bp + 1
        if bp not in act_tiles:
            load_pair(bp)
        if bp + 1 < NBP and bp + 1 not in act_tiles:
            load_pair(bp + 1)
        q2, da2 = act_tiles.pop(bp)

        # per-(b, qtile, k-chunk) collect tiles; finer chunks for the last
        # batch-pair so the final store flush is small
        KH = (K // 4) if bp < NBP - 1 else (K // 8)
        coll = {(h, qt_i, kh): out_pool.tile([128, KH, A], F32, name="coll", tag="coll")
                for h in (0, 1) for qt_i in range(QT) for kh in range(K // KH)}

        for k in range(K):
            ptmps = []
            for et in range(ET):
                pt = ptmp_pool.tile([128, 2 * Q], F32)
                for dc in range(DC):
                    nc.tensor.matmul(
                        pt[:],
                        lhsT=w_sb[k][:, dc, et * 128:(et + 1) * 128],
                        rhs=q2[:, dc, :],
                        start=(dc == 0),
                        stop=(dc == DC - 1),
                    )
                ptmps.append(pt)
            tmp = tmp_pool.tile([128, ET, 2 * Q], F16)
            for et in range(ET):
                nc.vector.tensor_scalar_add(
                    tmp[:, et, :], ptmps[et][:], vdt_sb[:, et, k:k + 1]
                )
            for h, b in ((0, b0), (1, b1)):
                for qt_i in range(QT):
                    po = pout_pool.tile([128, A], F32)
                    for et in range(ET):
                        nc.tensor.matmul(
                            po[:],
                            lhsT=tmp[:, et, h * Q + qt_i * 128: h * Q + (qt_i + 1) * 128],
                            rhs=da2[:, et, h * A:(h + 1) * A],
                            start=(et == 0),
                            stop=(et == ET - 1),
                        )
                    nc.scalar.activation(
                        coll[(h, qt_i, k // KH)][:, k % KH, :], po[:], SIG,
                        bias=mq_sb[:, qt_i, b, k:k + 1],
                    )
            if (k + 1) % KH == 0:
                kh = k // KH
                store_eng = nc.sync
                for h, b in ((0, b0), (1, b1)):
                    for qt_i in range(QT):
                        store_eng.dma_start(
                            out[b, kh * KH:(kh + 1) * KH,
                                qt_i * 128:(qt_i + 1) * 128, :].rearrange("k p a -> p k a"),
                            coll[(h, qt_i, kh)][:],
                        )


_COMPILED = None


def _get_compiled():
    global _COMPILED
    if _COMPILED is not None:
        return _COMPILED
    nc = bacc.Bacc("TRN2", target_bir_lowering=False, debug=False, num_devices=NCORES)
    aps = {
        "qt": nc.dram_tensor("qt", [BL, 128, D // 128, Q], F16, kind="ExternalInput").ap(),
        "dat": nc.dram_tensor("dat", [BL, 128, E // 128, A], F16, kind="ExternalInput").ap(),
        "w": nc.dram_tensor("w", [K, D, E], F16, kind="ExternalInput").ap(),
        "vdt": nc.dram_tensor("vdt", [E // 128, 128, 128], F32, kind="ExternalInput").ap(),
        "mq": nc.dram_tensor("mq", [Q // 128, 128, BL, K], F32, kind="ExternalInput").ap(),
        "out": nc.dram_tensor("out", [BL, K, Q, A], F32, kind="ExternalOutput").ap(),
    }
    with tile.TileContext(nc) as tc:
        with ExitStack() as ctx:
            _build_ntn(tc, ctx, aps)
    nc.compile()
    _COMPILED = nc
    return nc


def kernel(batch_q_em, batch_da_em, w, V, b):
    q = np.ascontiguousarray(np.asarray(batch_q_em, dtype=np.float32))
    da = np.ascontiguousarray(np.asarray(batch_da_em, dtype=np.float32))
    w = np.ascontiguousarray(np.asarray(w, dtype=np.float32))
    V = np.ascontiguousarray(np.asarray(V, dtype=np.float32))
    b = np.asarray(b, dtype=np.float32).reshape(-1)

    # packed to SBUF layout [b, p, dc, q] so each load is 128 x 1KB descriptors
    qt = np.ascontiguousarray(
        q.transpose(0, 2, 1).reshape(B, D // 128, 128, Q).transpose(0, 2, 1, 3)
    ).astype(np.float16)                                  # [B, 128, DC, Q]
    dat = np.ascontiguousarray(
        da.transpose(0, 2, 1).reshape(B, E // 128, 128, A).transpose(0, 2, 1, 3)
    ).astype(np.float16)                                  # [B, 128, ET, A]
    w16 = w.astype(np.float16)
    vdt_cols = np.ascontiguousarray(V[:, D:].T)          # [E, K]
    vdt = np.zeros((E // 128, 128, 128), dtype=np.float32)
    vdt[:, :, :K] = vdt_cols.reshape(E // 128, 128, K)
    # mq[b,q,k] = q[b] @ Vq^T + bias
    mqT = q @ V[:, :D].T + b[None, None, :]              # [B, Q, K]

    nc = _get_compiled()
    in_maps = []
    for c in range(NCORES):
        s = slice(c * BL, (c + 1) * BL)
        mq_shard = np.ascontiguousarray(
            mqT[s].reshape(BL, Q // 128, 128, K).transpose(1, 2, 0, 3)
        )  # [QT, 128, BL, K]
        in_maps.append({
            "qt": np.ascontiguousarray(qt[s]),
            "dat": np.ascontiguousarray(dat[s]),
            "w": w16,
            "vdt": vdt,
            "mq": mq_shard,
        })

    trace = bool(int(os.environ.get("NTN_TRACE", "0"))) and _install_profshim()
    res = bass_utils.run_bass_kernel_spmd(
        nc, in_maps, core_ids=list(range(NCORES)), trace=trace
    )
    if trace and res.exec_time_ns is not None:
        print(f"HW exec time: {res.exec_time_ns} ns")
    out = np.concatenate([r["out"] for r in res.results], axis=0)
    return out


# revision 24
# speedup vs baseline: 1.0242x; 1.0055x over previous
"""Trainium2 Bass kernel for the NTN problem.

out[b,k,q,a] = sigmoid( q[b,q,:] @ w[k] @ da[b,a,:]
                        + Vq[k]@q[b,q,:] + Vd[k]@da[b,a,:] + b[k] )

B=64, K=16, Q=A=D=256.  Sharding: data-parallel over batch B across the
8 NeuronCores (8 batches per core); w/V/b replicated.

Per core, per (k, batch-pair):
  MM1 (TensorE, fp16): tmp[e, q|q'] = sum_d w[k,d,e]^T qT[d, q|q']   (N=512)
  DVE: tmp PSUM->SBUF (fp16) with per-partition bias +Vd[k,e] (folds Vd@da)
  MM2 (TensorE, fp16): out[q, a] = sum_e tmp[e,q]^T daT[e, a]
  ScalarE: sigmoid(psum + bias mq[b,k,q]) where mq = Vq@q + b (host-prepped),
           written into a per-(b,qtile) collect tile covering all 16 k
  One 2 MB DMA per (b, qtile) collect tile -> 16 output stores total.
"""

import os
import sys
import types
from contextlib import ExitStack

if "/opt/trn_rl_repo" not in sys.path:
    sys.path.insert(0, "/opt/trn_rl_repo")

import numpy as np

import concourse.bass as bass
import concourse.tile as tile
from concourse import bacc, bass_utils, mybir

F32 = mybir.dt.float32
F16 = mybir.dt.float16
SIG = mybir.ActivationFunctionType.Sigmoid

NCORES = 8
B, Q, A, D, K = 64, 256, 256, 256, 16
E = D
BL = B // NCORES


def _install_profshim():
    """Provide antenv.axon_hooks so trace=True works under axon (best-effort)."""
    try:
        if "antenv.axon_hooks" in sys.modules:
            return True
        import antenv

        mod = types.ModuleType("antenv.axon_hooks")
        holder = {}
        mod.set_axon_ntff_profile_hook = lambda h: holder.__setitem__("h", h)
        mod.get_axon_ntff_profile_hook = lambda: holder.get("h")
        sys.modules["antenv.axon_hooks"] = mod
        antenv.axon_hooks = mod
        from trn_agent_boot.trn_boot import _ntff_profile_via_ctypes

        hook = _ntff_profile_via_ctypes("/opt/axon/libaxon_pjrt.so")
        if hook is None:
            return False
        mod.set_axon_ntff_profile_hook(hook)
        return True
    except Exception:
        return False


def _build_ntn(tc: tile.TileContext, ctx: ExitStack, aps: dict):
    nc = tc.nc
    DC, ET, QT = D // 128, E // 128, Q // 128
    qt, dat, w, vdt, mq, out = (aps[n] for n in ("qt", "dat", "w", "vdt", "mq", "out"))

    w_pool = ctx.enter_context(tc.tile_pool(name="w", bufs=1))
    const_pool = ctx.enter_context(tc.tile_pool(name="const", bufs=1))
    q_pool = ctx.enter_context(tc.tile_pool(name="q", bufs=3))
    da_pool = ctx.enter_context(tc.tile_pool(name="da", bufs=3))
    tmp_pool = ctx.enter_context(tc.tile_pool(name="tmp", bufs=4))
    out_pool = ctx.enter_context(tc.tile_pool(name="out", bufs=24))
    ptmp_pool = ctx.enter_context(tc.tile_pool(name="ptmp", bufs=3, space="PSUM"))
    pout_pool = ctx.enter_context(tc.tile_pool(name="pout", bufs=5, space="PSUM"))

    # First batch-pair's activations first, so MM1(bp0,k=0) can start early;
    # then per-k w tiles, then the small constants.
    act_tiles = {}

    # PE warm-up: dummy matmuls on a zeroed scratch tile while input DMAs are
    # in flight, so HAM reaches full clock before the real stream starts.
    warm_pool = ctx.enter_context(tc.tile_pool(name="warm", bufs=1))
    scratch = warm_pool.tile([128, 512], F16)
    nc.gpsimd.memset(scratch[:], 0.0)
    pwarm = ptmp_pool.tile([128, 512], F32, name="pwarm", tag="pt")
    for _ in range(9):
        nc.tensor.matmul(pwarm[:], lhsT=scratch[:, 0:128], rhs=scratch[:],
                         start=True, stop=True)

    def load_pair(bp, first=False):
        b0, b1 = 2 * bp, 2 * bp + 1
        q2 = q_pool.tile([128, DC, 2 * Q], F16, name=f"q2_{bp}", tag="q2")
        nc.sync.dma_start(q2[:, :, 0:Q], qt[b0])
        nc.sync.dma_start(q2[:, :, Q:2 * Q], qt[b1])
        if first:
            return (q2, b0, b1)
        da2 = da_pool.tile([128, ET, 2 * A], F16, name=f"da2_{bp}", tag="da2")
        nc.sync.dma_start(da2[:, :, 0:A], dat[b0])
        nc.sync.dma_start(da2[:, :, A:2 * A], dat[b1])
        act_tiles[bp] = (q2, da2)

    def load_wk(k):
        wk = w_pool.tile([128, DC, E], F16, name=f"wk{k}", tag=f"wk{k}")
        nc.sync.dma_start(wk[:], w[k].rearrange("(dc p) e -> p dc e", p=128))
        return wk

    # critical first loads on ScalarE's HWDGE stream (its framework preamble
    # ends ~1.7us before SP's, so these issue earlier)
    b0_0, b1_0 = 0, 1
    q2_0 = q_pool.tile([128, DC, 2 * Q], F16, name="q2_0", tag="q2")
    nc.scalar.dma_start(q2_0[:, :, 0:Q], qt[b0_0])
    nc.scalar.dma_start(q2_0[:, :, Q:2 * Q], qt[b1_0])
    w_sb = {}
    wk0 = w_pool.tile([128, DC, E], F16, name="wk0", tag="wk0")
    nc.scalar.dma_start(wk0[:], w[0].rearrange("(dc p) e -> p dc e", p=128))
    w_sb[0] = wk0
    mq_sb = const_pool.tile([128, QT, BL, K], F32)
    nc.sync.dma_start(mq_sb[:], mq.rearrange("t p b k -> p t b k"))
    da2_0 = da_pool.tile([128, ET, 2 * A], F16, name="da2_0", tag="da2")
    nc.sync.dma_start(da2_0[:, :, 0:A], dat[b0_0])
    nc.sync.dma_start(da2_0[:, :, A:2 * A], dat[b1_0])
    act_tiles[0] = (q2_0, da2_0)
    w_sb[1] = load_wk(1)
    vdt_sb = const_pool.tile([128, ET, 128], F32)
    nc.sync.dma_start(vdt_sb[:], vdt.rearrange("et p k -> p et k"))
    for k in range(2, K):
        w_sb[k] = load_wk(k)

    NBP = BL // 2
    for bp in range(NBP):
        b0, b1 = 2 * bp, 2 * bp + 1
        if bp not in act_tiles:
            load_pair(bp)
        if bp + 1 < NBP and bp + 1 not in act_tiles:
            load_pair(bp + 1)
        q2, da2 = act_tiles.pop(bp)

        # per-(b, qtile, k-chunk) collect tiles; chunks shrink toward the end
        # of the last batch-pair so the final store flush is small
        if bp < NBP - 1:
            chunk_sizes = [4, 4, 4, 4]
        else:
            chunk_sizes = [2, 2, 2, 2, 2, 2, 2, 1, 1]
        k2chunk = {}
        koff = 0
        for ci, cs in enumerate(chunk_sizes):
            for off in range(cs):
                k2chunk[koff + off] = (ci, off, cs)
            koff += cs
        coll = {(h, qt_i, ci): out_pool.tile([128, cs, A], F32, name="coll", tag="coll")
                for h in (0, 1) for qt_i in range(QT)
                for ci, cs in enumerate(chunk_sizes)}

        for k in range(K):
            ptmps = []
            for et in range(ET):
                pt = ptmp_pool.tile([128, 2 * Q], F32)
                for dc in range(DC):
                    nc.tensor.matmul(
                        pt[:],
                        lhsT=w_sb[k][:, dc, et * 128:(et + 1) * 128],
                        rhs=q2[:, dc, :],
                        start=(dc == 0),
                        stop=(dc == DC - 1),
                    )
                ptmps.append(pt)
            tmp = tmp_pool.tile([128, ET, 2 * Q], F16)
            for et in range(ET):
                nc.vector.tensor_scalar_add(
                    tmp[:, et, :], ptmps[et][:], vdt_sb[:, et, k:k + 1]
                )
            for h, b in ((0, b0), (1, b1)):
                for qt_i in range(QT):
                    po = pout_pool.tile([128, A], F32)
                    for et in range(ET):
                        nc.tensor.matmul(
                            po[:],
                            lhsT=tmp[:, et, h * Q + qt_i * 128: h * Q + (qt_i + 1) * 128],
                            rhs=da2[:, et, h * A:(h + 1) * A],
                            start=(et == 0),
                            stop=(et == ET - 1),
                        )
                    nc.scalar.activation(
                        coll[(h, qt_i, k2chunk[k][0])][:, k2chunk[k][1], :], po[:], SIG,
                        bias=mq_sb[:, qt_i, b, k:k + 1],
                    )
            ci, off, cs = k2chunk[k]
            if off == cs - 1:
                k_lo = k - cs + 1
                last_chunks = bp == NBP - 1 and ci >= len(chunk_sizes) - 2
                for idx, (h, b) in enumerate(((0, b0), (1, b1))):
                    for qt_i in range(QT):
                        # final flush: split issue across SP and ScalarE (whose
                        # queue has drained by then) to halve serialization
                        eng = nc.scalar if (last_chunks and idx == 1) else nc.sync
                        eng.dma_start(
                            out[b, k_lo:k_lo + cs,
                                qt_i * 128:(qt_i + 1) * 128, :].rearrange("k p a -> p k a"),
                            coll[(h, qt_i, ci)][:],
                        )


_COMPILED = None


def _get_compiled():
    global _COMPILED
    if _COMPILED is not None:
        return _COMPILED
    nc = bacc.Bacc("TRN2", target_bir_lowering=False, debug=False, num_devices=NCORES)
    aps = {
        "qt": nc.dram_tensor("qt", [BL, 128, D // 128, Q], F16, kind="ExternalInput").ap(),
        "dat": nc.dram_tensor("dat", [BL, 128, E // 128, A], F16, kind="ExternalInput").ap(),
        "w": nc.dram_tensor("w", [K, D, E], F16, kind="ExternalInput").ap(),
        "vdt": nc.dram_tensor("vdt", [E // 128, 128, 128], F32, kind="ExternalInput").ap(),
        "mq": nc.dram_tensor("mq", [Q // 128, 128, BL, K], F32, kind="ExternalInput").ap(),
        "out": nc.dram_tensor("out", [BL, K, Q, A], F32, kind="ExternalOutput").ap(),
    }
    with tile.TileContext(nc) as tc:
        with ExitStack() as ctx:
            _build_ntn(tc, ctx, aps)
    nc.compile()
    _COMPILED = nc
    return nc


def kernel(batch_q_em, batch_da_em, w, V, b):
    q = np.ascontiguousarray(np.asarray(batch_q_em, dtype=np.float32))
    da = np.ascontiguousarray(np.asarray(batch_da_em, dtype=np.float32))
    w = np.ascontiguousarray(np.asarray(w, dtype=np.float32))
    V = np.ascontiguousarray(np.asarray(V, dtype=np.float32))
    b = np.asarray(b, dtype=np.float32).reshape(-1)

    # packed to SBUF layout [b, p, dc, q] so each load is 128 x 1KB descriptors
    qt = np.ascontiguousarray(
        q.transpose(0, 2, 1).reshape(B, D // 128, 128, Q).transpose(0, 2, 1, 3)
    ).astype(np.float16)                                  # [B, 128, DC, Q]
    dat = np.ascontiguousarray(
        da.transpose(0, 2, 1).reshape(B, E // 128, 128, A).transpose(0, 2, 1, 3)
    ).astype(np.float16)                                  # [B, 128, ET, A]
    w16 = w.astype(np.float16)
    vdt_cols = np.ascontiguousarray(V[:, D:].T)          # [E, K]
    vdt = np.zeros((E // 128, 128, 128), dtype=np.float32)
    vdt[:, :, :K] = vdt_cols.reshape(E // 128, 128, K)
    # mq[b,q,k] = q[b] @ Vq^T + bias
    mqT = q @ V[:, :D].T + b[None, None, :]              # [B, Q, K]

    nc = _get_compiled()
    in_maps = []
    for c in range(NCORES):
        s = slice(c * BL, (c + 1) * BL)
        mq_shard = np.ascontiguousarray(
            mqT[s].reshape(BL, Q // 128, 128, K).transpose(1, 2, 0, 3)
        )  # [QT, 128, BL, K]
        in_maps.append({
            "qt": np.ascontiguousarray(qt[s]),
            "dat": np.ascontiguousarray(dat[s]),
            "w": w16,
            "vdt": vdt,
            "mq": mq_shard,
        })

    trace = bool(int(os.environ.get("NTN_TRACE", "0"))) and _install_profshim()
    res = bass_utils.run_bass_kernel_spmd(
        nc, in_maps, core_ids=list(range(NCORES)), trace=trace
    )
    if trace and res.exec_time_ns is not None:
        print(f"HW exec time: {res.exec_time_ns} ns")
    out = np.concatenate([r["out"] for r in res.results], axis=0)
    return out


# revision 25
# speedup vs baseline: 1.0244x; 1.0003x over previous
"""Trainium2 Bass kernel for the NTN problem.

out[b,k,q,a] = sigmoid( q[b,q,:] @ w[k] @ da[b,a,:]
                        + Vq[k]@q[b,q,:] + Vd[k]@da[b,a,:] + b[k] )

B=64, K=16, Q=A=D=256.  Sharding: data-parallel over batch B across the
8 NeuronCores (8 batches per core); w/V/b replicated.

Per core, per (k, batch-pair):
  MM1 (TensorE, fp16): tmp[e, q|q'] = sum_d w[k,d,e]^T qT[d, q|q']   (N=512)
  DVE: tmp PSUM->SBUF (fp16) with per-partition bias +Vd[k,e] (folds Vd@da)
  MM2 (TensorE, fp16): out[q, a] = sum_e tmp[e,q]^T daT[e, a]
  ScalarE: sigmoid(psum + bias mq[b,k,q]) where mq = Vq@q + b (host-prepped),
           written into a per-(b,qtile) collect tile covering all 16 k
  One 2 MB DMA per (b, qtile) collect tile -> 16 output stores total.
"""

import os
import sys
import types
from contextlib import ExitStack

if "/opt/trn_rl_repo" not in sys.path:
    sys.path.insert(0, "/opt/trn_rl_repo")

import numpy as np

import concourse.bass as bass
import concourse.tile as tile
from concourse import bacc, bass_utils, mybir

F32 = mybir.dt.float32
F16 = mybir.dt.float16
SIG = mybir.ActivationFunctionType.Sigmoid

NCORES = 8
B, Q, A, D, K = 64, 256, 256, 256, 16
E = D
BL = B // NCORES


def _install_profshim():
    """Provide antenv.axon_hooks so trace=True works under axon (best-effort)."""
    try:
        if "antenv.axon_hooks" in sys.modules:
            return True
        import antenv

        mod = types.ModuleType("antenv.axon_hooks")
        holder = {}
        mod.set_axon_ntff_profile_hook = lambda h: holder.__setitem__("h", h)
        mod.get_axon_ntff_profile_hook = lambda: holder.get("h")
        sys.modules["antenv.axon_hooks"] = mod
        antenv.axon_hooks = mod
        from trn_agent_boot.trn_boot import _ntff_profile_via_ctypes

        hook = _ntff_profile_via_ctypes("/opt/axon/libaxon_pjrt.so")
        if hook is None:
            return False
        mod.set_axon_ntff_profile_hook(hook)
        return True
    except Exception:
        return False


def _build_ntn(tc: tile.TileContext, ctx: ExitStack, aps: dict):
    nc = tc.nc
    DC, ET, QT = D // 128, E // 128, Q // 128
    qt, dat, w, vdt, mq, out = (aps[n] for n in ("qt", "dat", "w", "vdt", "mq", "out"))

    w_pool = ctx.enter_context(tc.tile_pool(name="w", bufs=1))
    const_pool = ctx.enter_context(tc.tile_pool(name="const", bufs=1))
    q_pool = ctx.enter_context(tc.tile_pool(name="q", bufs=3))
    da_pool = ctx.enter_context(tc.tile_pool(name="da", bufs=3))
    tmp_pool = ctx.enter_context(tc.tile_pool(name="tmp", bufs=4))
    out_pool = ctx.enter_context(tc.tile_pool(name="out", bufs=24))
    ptmp_pool = ctx.enter_context(tc.tile_pool(name="ptmp", bufs=3, space="PSUM"))
    pout_pool = ctx.enter_context(tc.tile_pool(name="pout", bufs=5, space="PSUM"))

    # First batch-pair's activations first, so MM1(bp0,k=0) can start early;
    # then per-k w tiles, then the small constants.
    act_tiles = {}

    # PE warm-up: dummy matmuls on a zeroed scratch tile while input DMAs are
    # in flight, so HAM reaches full clock before the real stream starts.
    warm_pool = ctx.enter_context(tc.tile_pool(name="warm", bufs=1))
    scratch = warm_pool.tile([128, 512], F16)
    nc.gpsimd.memset(scratch[:], 0.0)
    pwarm = ptmp_pool.tile([128, 512], F32, name="pwarm", tag="pt")
    for _ in range(8):
        nc.tensor.matmul(pwarm[:], lhsT=scratch[:, 0:128], rhs=scratch[:],
                         start=True, stop=True)

    def load_pair(bp, first=False):
        b0, b1 = 2 * bp, 2 * bp + 1
        q2 = q_pool.tile([128, DC, 2 * Q], F16, name=f"q2_{bp}", tag="q2")
        nc.sync.dma_start(q2[:].rearrange("p dc (h q) -> p dc h q", h=2),
                          qt[b0:b0 + 2].rearrange("h p dc q -> p dc h q"))
        if first:
            return (q2, b0, b1)
        da2 = da_pool.tile([128, ET, 2 * A], F16, name=f"da2_{bp}", tag="da2")
        nc.sync.dma_start(da2[:].rearrange("p et (h a) -> p et h a", h=2),
                          dat[b0:b0 + 2].rearrange("h p et a -> p et h a"))
        act_tiles[bp] = (q2, da2)

    def load_wk(k):
        wk = w_pool.tile([128, DC, E], F16, name=f"wk{k}", tag=f"wk{k}")
        nc.sync.dma_start(wk[:], w[k].rearrange("(dc p) e -> p dc e", p=128))
        return wk

    # critical first loads on ScalarE's HWDGE stream (its framework preamble
    # ends ~1.7us before SP's, so these issue earlier)
    b0_0, b1_0 = 0, 1
    q2_0 = q_pool.tile([128, DC, 2 * Q], F16, name="q2_0", tag="q2")
    nc.scalar.dma_start(q2_0[:].rearrange("p dc (h q) -> p dc h q", h=2),
                        qt[0:2].rearrange("h p dc q -> p dc h q"))
    w_sb = {}
    wk0 = w_pool.tile([128, DC, E], F16, name="wk0", tag="wk0")
    nc.scalar.dma_start(wk0[:], w[0].rearrange("(dc p) e -> p dc e", p=128))
    w_sb[0] = wk0
    mq_sb = const_pool.tile([128, QT, BL, K], F32)
    nc.sync.dma_start(mq_sb[:], mq.rearrange("t p b k -> p t b k"))
    da2_0 = da_pool.tile([128, ET, 2 * A], F16, name="da2_0", tag="da2")
    nc.sync.dma_start(da2_0[:].rearrange("p et (h a) -> p et h a", h=2),
                      dat[0:2].rearrange("h p et a -> p et h a"))
    act_tiles[0] = (q2_0, da2_0)
    w_sb[1] = load_wk(1)
    vdt_sb = const_pool.tile([128, ET, 128], F32)
    nc.sync.dma_start(vdt_sb[:], vdt.rearrange("et p k -> p et k"))
    for k in range(2, K):
        w_sb[k] = load_wk(k)

    NBP = BL // 2
    for bp in range(NBP):
        b0, b1 = 2 * bp, 2 * bp + 1
        if bp not in act_tiles:
            load_pair(bp)
        if bp + 1 < NBP and bp + 1 not in act_tiles:
            load_pair(bp + 1)
        q2, da2 = act_tiles.pop(bp)

        # per-(b, qtile, k-chunk) collect tiles; chunks shrink toward the end
        # of the last batch-pair so the final store flush is small
        if bp < NBP - 1:
            chunk_sizes = [4, 4, 4, 4]
        else:
            chunk_sizes = [2, 2, 2, 2, 2, 2, 2, 1, 1]
        k2chunk = {}
        koff = 0
        for ci, cs in enumerate(chunk_sizes):
            for off in range(cs):
                k2chunk[koff + off] = (ci, off, cs)
            koff += cs
        coll = {(h, qt_i, ci): out_pool.tile([128, cs, A], F32, name="coll", tag="coll")
                for h in (0, 1) for qt_i in range(QT)
                for ci, cs in enumerate(chunk_sizes)}

        for k in range(K):
            ptmps = []
            for et in range(ET):
                pt = ptmp_pool.tile([128, 2 * Q], F32)
                for dc in range(DC):
                    nc.tensor.matmul(
                        pt[:],
                        lhsT=w_sb[k][:, dc, et * 128:(et + 1) * 128],
                        rhs=q2[:, dc, :],
                        start=(dc == 0),
                        stop=(dc == DC - 1),
                    )
                ptmps.append(pt)
            tmp = tmp_pool.tile([128, ET, 2 * Q], F16)
            for et in range(ET):
                nc.vector.tensor_scalar_add(
                    tmp[:, et, :], ptmps[et][:], vdt_sb[:, et, k:k + 1]
                )
            for h, b in ((0, b0), (1, b1)):
                for qt_i in range(QT):
                    po = pout_pool.tile([128, A], F32)
                    for et in range(ET):
                        nc.tensor.matmul(
                            po[:],
                            lhsT=tmp[:, et, h * Q + qt_i * 128: h * Q + (qt_i + 1) * 128],
                            rhs=da2[:, et, h * A:(h + 1) * A],
                            start=(et == 0),
                            stop=(et == ET - 1),
                        )
                    nc.scalar.activation(
                        coll[(h, qt_i, k2chunk[k][0])][:, k2chunk[k][1], :], po[:], SIG,
                        bias=mq_sb[:, qt_i, b, k:k + 1],
                    )
            ci, off, cs = k2chunk[k]
            if off == cs - 1:
                k_lo = k - cs + 1
                last_chunks = bp == NBP - 1 and ci >= len(chunk_sizes) - 2
                for idx, (h, b) in enumerate(((0, b0), (1, b1))):
                    for qt_i in range(QT):
                        # final flush: split issue across SP and ScalarE (whose
                        # queue has drained by then) to halve serialization
                        eng = nc.scalar if (last_chunks and idx == 1) else nc.sync
                        eng.dma_start(
                            out[b, k_lo:k_lo + cs,
                                qt_i * 128:(qt_i + 1) * 128, :].rearrange("k p a -> p k a"),
                            coll[(h, qt_i, ci)][:],
                        )


_COMPILED = None


def _get_compiled():
    global _COMPILED
    if _COMPILED is not None:
        return _COMPILED
    nc = bacc.Bacc("TRN2", target_bir_lowering=False, debug=False, num_devices=NCORES)
    aps = {
        "qt": nc.dram_tensor("qt", [BL, 128, D // 128, Q], F16, kind="ExternalInput").ap(),
        "dat": nc.dram_tensor("dat", [BL, 128, E // 128, A], F16, kind="ExternalInput").ap(),
        "w": nc.dram_tensor("w", [K, D, E], F16, kind="ExternalInput").ap(),
        "vdt": nc.dram_tensor("vdt", [E // 128, 128, 128], F32, kind="ExternalInput").ap(),
        "mq": nc.dram_tensor("mq", [Q // 128, 128, BL, K], F32, kind="ExternalInput").ap(),
        "out": nc.dram_tensor("out", [BL, K, Q, A], F32, kind="ExternalOutput").ap(),
    }
    with tile.TileContext(nc) as tc:
        with ExitStack() as ctx:
            _build_ntn(tc, ctx, aps)
    nc.compile()
    _COMPILED = nc
    return nc


def kernel(batch_q_em, batch_da_em, w, V, b):
    q = np.ascontiguousarray(np.asarray(batch_q_em, dtype=np.float32))
    da = np.ascontiguousarray(np.asarray(batch_da_em, dtype=np.float32))
    w = np.ascontiguousarray(np.asarray(w, dtype=np.float32))
    V = np.ascontiguousarray(np.asarray(V, dtype=np.float32))
    b = np.asarray(b, dtype=np.float32).reshape(-1)

    # packed to SBUF layout [b, p, dc, q] so each load is 128 x 1KB descriptors
    qt = np.ascontiguousarray(
        q.transpose(0, 2, 1).reshape(B, D // 128, 128, Q).transpose(0, 2, 1, 3)
    ).astype(np.float16)                                  # [B, 128, DC, Q]
    dat = np.ascontiguousarray(
        da.transpose(0, 2, 1).reshape(B, E // 128, 128, A).transpose(0, 2, 1, 3)
    ).astype(np.float16)                                  # [B, 128, ET, A]
    w16 = w.astype(np.float16)
    vdt_cols = np.ascontiguousarray(V[:, D:].T)          # [E, K]
    vdt = np.zeros((E // 128, 128, 128), dtype=np.float32)
    vdt[:, :, :K] = vdt_cols.reshape(E // 128, 128, K)
    # mq[b,q,k] = q[b] @ Vq^T + bias
    mqT = q @ V[:, :D].T + b[None, None, :]              # [B, Q, K]

    nc = _get_compiled()
    in_maps = []
    for c in range(NCORES):
        s = slice(c * BL, (c + 1) * BL)
        mq_shard = np.ascontiguousarray(
            mqT[s].reshape(BL, Q // 128, 128, K).transpose(1, 2, 0, 3)
        )  # [QT, 128, BL, K]
        in_maps.append({
            "qt": np.ascontiguousarray(qt[s]),
            "dat": np.ascontiguousarray(dat[s]),
            "w": w16,
            "vdt": vdt,
            "mq": mq_shard,
        })

    trace = bool(int(os.environ.get("NTN_TRACE", "0"))) and _install_profshim()
    res = bass_utils.run_bass_kernel_spmd(
        nc, in_maps, core_ids=list(range(NCORES)), trace=trace
    )
    if trace and res.exec_time_ns is not None:
        print(f"HW exec time: {res.exec_time_ns} ns")
    out = np.concatenate([r["out"] for r in res.results], axis=0)
    return out


# revision 27
# speedup vs baseline: 1.0364x; 1.0117x over previous
"""Trainium2 Bass kernel for the NTN problem.

out[b,k,q,a] = sigmoid( q[b,q,:] @ w[k] @ da[b,a,:]
                        + Vq[k]@q[b,q,:] + Vd[k]@da[b,a,:] + b[k] )

B=64, K=16, Q=A=D=256.  Sharding: data-parallel over batch B across the
8 NeuronCores (8 batches per core); w/V/b replicated.

Per core, per (k, batch-pair):
  MM1 (TensorE, fp16): tmp[e, q|q'] = sum_d w[k,d,e]^T qT[d, q|q']   (N=512)
  DVE: tmp PSUM->SBUF (fp16) with per-partition bias +Vd[k,e] (folds Vd@da)
  MM2 (TensorE, fp16): out[q, a] = sum_e tmp[e,q]^T daT[e, a]
  ScalarE: sigmoid(psum + bias mq[b,k,q]) where mq = Vq@q + b (host-prepped),
           written into a per-(b,qtile) collect tile covering all 16 k
  One 2 MB DMA per (b, qtile) collect tile -> 16 output stores total.
"""

import os
import sys
import types
from contextlib import ExitStack

if "/opt/trn_rl_repo" not in sys.path:
    sys.path.insert(0, "/opt/trn_rl_repo")

import numpy as np

import concourse.bass as bass
import concourse.tile as tile
from concourse import bacc, bass_utils, mybir

F32 = mybir.dt.float32
F16 = mybir.dt.float16
SIG = mybir.ActivationFunctionType.Sigmoid

NCORES = 8
B, Q, A, D, K = 64, 256, 256, 256, 16
E = D
BL = B // NCORES


def _install_profshim():
    """Provide antenv.axon_hooks so trace=True works under axon (best-effort)."""
    try:
        if "antenv.axon_hooks" in sys.modules:
            return True
        import antenv

        mod = types.ModuleType("antenv.axon_hooks")
        holder = {}
        mod.set_axon_ntff_profile_hook = lambda h: holder.__setitem__("h", h)
        mod.get_axon_ntff_profile_hook = lambda: holder.get("h")
        sys.modules["antenv.axon_hooks"] = mod
        antenv.axon_hooks = mod
        from trn_agent_boot.trn_boot import _ntff_profile_via_ctypes

        hook = _ntff_profile_via_ctypes("/opt/axon/libaxon_pjrt.so")
        if hook is None:
            return False
        mod.set_axon_ntff_profile_hook(hook)
        return True
    except Exception:
        return False


def _build_ntn(tc: tile.TileContext, ctx: ExitStack, aps: dict):
    nc = tc.nc
    DC, ET, QT = D // 128, E // 128, Q // 128
    qt, dat, w, vdt, mq, out = (aps[n] for n in ("qt", "dat", "w", "vdt", "mq", "out"))

    w_pool = ctx.enter_context(tc.tile_pool(name="w", bufs=1))
    const_pool = ctx.enter_context(tc.tile_pool(name="const", bufs=1))
    q_pool = ctx.enter_context(tc.tile_pool(name="q", bufs=3))
    da_pool = ctx.enter_context(tc.tile_pool(name="da", bufs=3))
    tmp_pool = ctx.enter_context(tc.tile_pool(name="tmp", bufs=4))
    out_pool = ctx.enter_context(tc.tile_pool(name="out", bufs=24))
    ptmp_pool = ctx.enter_context(tc.tile_pool(name="ptmp", bufs=3, space="PSUM"))
    pout_pool = ctx.enter_context(tc.tile_pool(name="pout", bufs=5, space="PSUM"))

    # First batch-pair's activations first, so MM1(bp0,k=0) can start early;
    # then per-k w tiles, then the small constants.
    act_tiles = {}

    # PE warm-up: dummy matmuls on a zeroed scratch tile while input DMAs are
    # in flight, so HAM reaches full clock before the real stream starts.
    warm_pool = ctx.enter_context(tc.tile_pool(name="warm", bufs=1))
    scratch = warm_pool.tile([128, 512], F16)
    nc.gpsimd.memset(scratch[:], 0.0)
    pwarm = ptmp_pool.tile([128, 512], F32, name="pwarm", tag="pt")
    for _ in range(8):
        nc.tensor.matmul(pwarm[:], lhsT=scratch[:, 0:128], rhs=scratch[:],
                         start=True, stop=True)

    def load_pair(bp, first=False):
        b0, b1 = 2 * bp, 2 * bp + 1
        q2 = q_pool.tile([128, DC, 2 * Q], F16, name=f"q2_{bp}", tag="q2")
        nc.sync.dma_start(q2[:].rearrange("p dc (h q) -> p dc h q", h=2),
                          qt[b0:b0 + 2].rearrange("h p dc q -> p dc h q"))
        if first:
            return (q2, b0, b1)
        da2 = da_pool.tile([128, ET, 2 * A], F16, name=f"da2_{bp}", tag="da2")
        nc.sync.dma_start(da2[:].rearrange("p et (h a) -> p et h a", h=2),
                          dat[b0:b0 + 2].rearrange("h p et a -> p et h a"))
        act_tiles[bp] = (q2, da2)

    def load_wk(k):
        wk = w_pool.tile([128, DC, E], F16, name=f"wk{k}", tag=f"wk{k}")
        nc.sync.dma_start(wk[:], w[k].rearrange("(dc p) e -> p dc e", p=128))
        return wk

    # critical first loads on ScalarE's HWDGE stream (its framework preamble
    # ends ~1.7us before SP's, so these issue earlier)
    b0_0, b1_0 = 0, 1
    q2_0 = q_pool.tile([128, DC, 2 * Q], F16, name="q2_0", tag="q2")
    nc.scalar.dma_start(q2_0[:].rearrange("p dc (h q) -> p dc h q", h=2),
                        qt[0:2].rearrange("h p dc q -> p dc h q"))
    w_sb = {}
    wk0 = w_pool.tile([128, DC, E], F16, name="wk0", tag="wk0")
    nc.scalar.dma_start(wk0[:], w[0].rearrange("(dc p) e -> p dc e", p=128))
    w_sb[0] = wk0
    mq_sb = const_pool.tile([128, QT, BL, K], F32)
    nc.sync.dma_start(mq_sb[:], mq.rearrange("t p b k -> p t b k"))
    da2_0 = da_pool.tile([128, ET, 2 * A], F16, name="da2_0", tag="da2")
    nc.sync.dma_start(da2_0[:].rearrange("p et (h a) -> p et h a", h=2),
                      dat[0:2].rearrange("h p et a -> p et h a"))
    act_tiles[0] = (q2_0, da2_0)
    w_sb[1] = load_wk(1)
    vdt_sb = const_pool.tile([128, ET, 128], F32)
    nc.sync.dma_start(vdt_sb[:], vdt.rearrange("et p k -> p et k"))
    for k in range(2, K):
        w_sb[k] = load_wk(k)

    NBP = BL // 2
    for bp in range(NBP):
        b0, b1 = 2 * bp, 2 * bp + 1
        if bp not in act_tiles:
            load_pair(bp)
        if bp + 1 < NBP and bp + 1 not in act_tiles:
            load_pair(bp + 1)
        q2, da2 = act_tiles.pop(bp)

        # per-(b, qtile, k-chunk) collect tiles; chunks shrink toward the end
        # of the last batch-pair so the final store flush is small
        if bp < NBP - 1:
            chunk_sizes = [4, 4, 4, 4]
        else:
            chunk_sizes = [2, 2, 2, 2, 2, 2, 2, 1, 1]
        k2chunk = {}
        koff = 0
        for ci, cs in enumerate(chunk_sizes):
            for off in range(cs):
                k2chunk[koff + off] = (ci, off, cs)
            koff += cs
        coll = {(h, qt_i, ci): out_pool.tile([128, cs, A], F32, name="coll", tag="coll")
                for h in (0, 1) for qt_i in range(QT)
                for ci, cs in enumerate(chunk_sizes)}

        for k in range(K):
            ptmps = []
            for et in range(ET):
                pt = ptmp_pool.tile([128, 2 * Q], F32)
                for dc in range(DC):
                    nc.tensor.matmul(
                        pt[:],
                        lhsT=w_sb[k][:, dc, et * 128:(et + 1) * 128],
                        rhs=q2[:, dc, :],
                        start=(dc == 0),
                        stop=(dc == DC - 1),
                    )
                ptmps.append(pt)
            tmp = tmp_pool.tile([128, ET, 2 * Q], F16)
            for et in range(ET):
                nc.vector.tensor_scalar_add(
                    tmp[:, et, :], ptmps[et][:], vdt_sb[:, et, k:k + 1]
                )
            for h, b in ((0, b0), (1, b1)):
                for qt_i in range(QT):
                    po = pout_pool.tile([128, A], F32)
                    for et in range(ET):
                        nc.tensor.matmul(
                            po[:],
                            lhsT=tmp[:, et, h * Q + qt_i * 128: h * Q + (qt_i + 1) * 128],
                            rhs=da2[:, et, h * A:(h + 1) * A],
                            start=(et == 0),
                            stop=(et == ET - 1),
                        )
                    nc.scalar.activation(
                        coll[(h, qt_i, k2chunk[k][0])][:, k2chunk[k][1], :], po[:], SIG,
                        bias=mq_sb[:, qt_i, b, k:k + 1],
                    )
            ci, off, cs = k2chunk[k]
            if off == cs - 1:
                k_lo = k - cs + 1
                last_chunks = bp == NBP - 1 and ci >= len(chunk_sizes) - 2
                for idx, (h, b) in enumerate(((0, b0), (1, b1))):
                    for qt_i in range(QT):
                        # final flush: split issue across SP and ScalarE (whose
                        # queue has drained by then) to halve serialization
                        eng = nc.scalar if (last_chunks and idx == 1) else nc.sync
                        eng.dma_start(
                            out[b, k_lo:k_lo + cs,
                                qt_i * 128:(qt_i + 1) * 128, :].rearrange("k p a -> p k a"),
                            coll[(h, qt_i, ci)][:],
                        )


_COMPILED = None


def _get_compiled():
    global _COMPILED
    if _COMPILED is not None:
        return _COMPILED
    nc = bacc.Bacc("TRN2", target_bir_lowering=False, debug=False, num_devices=NCORES)
    aps = {
        "qt": nc.dram_tensor("qt", [BL, 128, D // 128, Q], F16, kind="ExternalInput").ap(),
        "dat": nc.dram_tensor("dat", [BL, 128, E // 128, A], F16, kind="ExternalInput").ap(),
        "w": nc.dram_tensor("w", [K, D, E], F16, kind="ExternalInput").ap(),
        "vdt": nc.dram_tensor("vdt", [E // 128, 128, 128], F32, kind="ExternalInput").ap(),
        "mq": nc.dram_tensor("mq", [Q // 128, 128, BL, K], F32, kind="ExternalInput").ap(),
        "out": nc.dram_tensor("out", [BL, K, Q, A], F32, kind="ExternalOutput").ap(),
    }
    with tile.TileContext(nc) as tc:
        with ExitStack() as ctx:
            _build_ntn(tc, ctx, aps)
    nc.compile()
    _COMPILED = nc
    return nc


def kernel(batch_q_em, batch_da_em, w, V, b):
    q = np.ascontiguousarray(np.asarray(batch_q_em, dtype=np.float32))
    da = np.ascontiguousarray(np.asarray(batch_da_em, dtype=np.float32))
    w = np.ascontiguousarray(np.asarray(w, dtype=np.float32))
    V = np.ascontiguousarray(np.asarray(V, dtype=np.float32))
    b = np.asarray(b, dtype=np.float32).reshape(-1)

    # packed to SBUF layout [b, p, dc, q] so each load is 128 x 1KB descriptors
    qt = np.ascontiguousarray(
        q.transpose(0, 2, 1).reshape(B, D // 128, 128, Q).transpose(0, 2, 1, 3)
    ).astype(np.float16)                                  # [B, 128, DC, Q]
    dat = np.ascontiguousarray(
        da.transpose(0, 2, 1).reshape(B, E // 128, 128, A).transpose(0, 2, 1, 3)
    ).astype(np.float16)                                  # [B, 128, ET, A]
    w16 = w.astype(np.float16)
    vdt_cols = np.ascontiguousarray(V[:, D:].T)          # [E, K]
    vdt = np.zeros((E // 128, 128, 128), dtype=np.float32)
    vdt[:, :, :K] = vdt_cols.reshape(E // 128, 128, K)
    # mq[b,q,k] = q[b] @ Vq^T + bias
    mqT = q @ V[:, :D].T + b[None, None, :]              # [B, Q, K]

    nc = _get_compiled()
    in_maps = []
    for c in range(NCORES):
        s = slice(c * BL, (c + 1) * BL)
        mq_shard = np.ascontiguousarray(
            mqT[s].reshape(BL, Q // 128, 128, K).transpose(1, 2, 0, 3)
        )  # [QT, 128, BL, K]
        in_maps.append({
            "qt": np.ascontiguousarray(qt[s]),
            "dat": np.ascontiguousarray(dat[s]),
            "w": w16,
            "vdt": vdt,
            "mq": mq_shard,
        })

    trace = bool(int(os.environ.get("NTN_TRACE", "0"))) and _install_profshim()
    res = bass_utils.run_bass_kernel_spmd(
        nc, in_maps, core_ids=list(range(NCORES)), trace=trace
    )
    if trace and res.exec_time_ns is not None:
        print(f"HW exec time: {res.exec_time_ns} ns")
    out = np.concatenate([r["out"] for r in res.results], axis=0)
    return out
